# revision 15
# baseline (speedup 1.0000x reference)
"""AGThenGemm v2: data-parallel + Strassen on the fp16 part of GEMM2.

Per core r (B_LOCAL = 256):
  GEMM1: act_T[D, B_LOCAL] = W_prev^T @ A^T  (unchanged from v1), evicted to
         SBUF: rows 0..3071 fp16 (act_res), rows 3072..4095 e4m3 (act8_res).
  GEMM2: out[B_LOCAL, F] = act^T W_up.
    fp16 rows (0..3071): ONE level of Strassen over (B 256->128, D' 3072->1536,
    F 16384->8192): 7 products instead of 8 -> 12.5% fewer PE cycles. The five
    W-side combos (W11+W22 etc) are built on-device by the DVE from 4 plain
    W tiles per (j, kt); the five A-side combos are built once after GEMM1.
    fp8 rows (3072..4095): direct DoubleRow per output tile (as v1).
    Scale-matching: wu16 carries x2048 (exact pow2), wu8 = e4m3(W*2048),
    act8 = e4m3(act) -> every PSUM partial is at 2048x; evictions descale.
  PSUM: tag "wv" = [128, 4, 512] (4 banks) x bufs=2 = all 8 banks, shared by
    GEMM1 (4 mi subtiles) and the Strassen waves:
      wave A = M1,M2,M3,M4 | wave B = M5,M6,M7,F8_11 | wave C = F8_21,F8_12,F8_22
    Waves evict to SBUF fp16 (descale x2^-11); combines assemble
      C11 = M1+M4-M5+M7+F8_11   C12 = M3+M5+F8_12
      C21 = M2+M4+F8_21         C22 = M1-M2+M3+M6+F8_22
  Engine split: DVE = W-combos, evictA/B, combines (combines software-pipelined
  one j behind, emitted mid-q-stream); Scalar = evictC + wu8 loads; GpSimd =
  out DMA.
"""

from contextlib import ExitStack

import numpy as np
import ml_dtypes

import concourse.tile as tile
from concourse import bacc, mybir
from concourse.bass_utils import run_bass_kernel_spmd

N_CORES = 8
B, K_PREV, D, F = 2048, 4096, 4096, 16384
B_LOCAL = B // N_CORES

P = 128
DS = 3072          # fp16 D-rows in GEMM2 contraction
D8 = D - DS        # fp8 D-rows (DoubleRow)
PAIRS = D8 // 256  # DoubleRow instructions per (F-block, mi)
W_SCALE = 2048.0
COMBINE = 1.0 / W_SCALE

NJ = 16            # Strassen F-block pairs: block j covers out cols
                   # [j*512, +512) and [8192 + j*512, +512)
KT3 = 12           # 128-row chunks per Strassen D-half (1536/128)

F8 = ml_dtypes.float8_e4m3


def build_nc(debug=False):
    nc = bacc.Bacc(
        "TRN2",
        target_bir_lowering=False,
        debug=debug,
        num_devices=N_CORES,
    )
    dt = mybir.dt.float16
    f8 = mybir.dt.float8e4
    f32 = mybir.dt.float32
    ADD = mybir.AluOpType.add
    SUB = mybir.AluOpType.subtract

    KT1 = K_PREV // 512   # 8 k-tiles for GEMM1
    M1_TILES = D // 512   # 8 output-D tiles for GEMM1
    M1_SUB = 4            # 128-subtiles per 512 tile
    NBLKS = F // 512      # 32 F-col blocks of 512

    a_t = nc.dram_tensor("a_t", [KT1 * P, 4 * B_LOCAL], dt, kind="ExternalInput")
    w_prev = nc.dram_tensor("w_prev", [M1_TILES * KT1 * P, 4 * 512], dt, kind="ExternalInput")
    # Strassen W tiles: per (j, kt): [p, blk(4), 512] with blk = (w11,w12,w21,w22)
    wu16 = nc.dram_tensor("wu16", [NJ * KT3 * P, 4 * 512], dt, kind="ExternalInput")
    wu8 = nc.dram_tensor("wu8", [NBLKS * P, PAIRS * 2 * 512], f8, kind="ExternalInput")
    out = nc.dram_tensor("out", [B_LOCAL, F], dt, kind="ExternalOutput")

    a4 = a_t.ap().rearrange("(j p) (ki b) -> j p ki b", j=KT1, ki=4)
    wp5 = w_prev.ap().rearrange(
        "(mt kt p) (ki m) -> mt kt p ki m", mt=M1_TILES, kt=KT1, ki=4
    )
    wu16_5 = wu16.ap().rearrange(
        "(j kt p) (blk n) -> j kt p blk n", j=NJ, kt=KT3, blk=4
    )
    wu8_5 = wu8.ap().rearrange(
        "(nb p) (pr two n) -> nb p pr two n", nb=NBLKS, pr=PAIRS, two=2
    )
    out3 = out.ap().rearrange("(mo p) n -> p mo n", p=P)  # [P, 2, F]

    with tile.TileContext(nc) as tc:
        with ExitStack() as ctx:
            wp_pool = ctx.enter_context(tc.tile_pool(name="wp_pool", bufs=6))
            w2_pool = ctx.enter_context(tc.tile_pool(name="w2_pool", bufs=10))
            q_pool = ctx.enter_context(tc.tile_pool(name="q_pool", bufs=8))
            wu8_pool = ctx.enter_context(tc.tile_pool(name="wu8_pool", bufs=2))
            m_pool = ctx.enter_context(tc.tile_pool(name="m_pool", bufs=2))
            temps = ctx.enter_context(tc.tile_pool(name="temps", bufs=2))
            ot_pool = ctx.enter_context(tc.tile_pool(name="ot_pool", bufs=2))
            res_pool = ctx.enter_context(tc.tile_pool(name="res_pool", bufs=1))
            psum = ctx.enter_context(tc.tile_pool(name="psum", bufs=2, space="PSUM"))

            a_res = res_pool.tile([P, KT1, 4, B_LOCAL], dt, name="a_res")
            act_res = res_pool.tile([P, DS // P, B_LOCAL], dt, name="act_res")
            act8_res = res_pool.tile([P, PAIRS, 2, B_LOCAL], f8, name="act8_res")
            # A-side Winograd combos: [p, c, kt, 128] with
            # c: 0=S1=A21+A22, 1=S2=S1-A11, 2=S3=A11-A21, 3=S4=A12-S2
            sa_res = res_pool.tile([P, 4, KT3, P], dt, name="sa_res")

            nc.scalar.dma_start(a_res[:, 0, 0:1, :], a4[0, :, 0:1, :])
            nc.scalar.dma_start(a_res[:, 0, 1:2, :], a4[0, :, 1:2, :])
            nc.scalar.dma_start(a_res[:, 0, 2:4, :], a4[0, :, 2:4, :])

            # ---------------- GEMM1 ----------------
            # PSUM budget: "wv" = [P,3,512] (3 banks) x2 bufs + "f8" =
            # [P,512] (1 bank) x2 bufs = 8 banks. GEMM1 puts mi 0..2 in a wv
            # tile and mi 3 in an f8 tile.
            for mt in range(M1_TILES):
                wv = psum.tile([P, 3, 512], f32, name="g1", tag="wv")
                g1b = psum.tile([P, 512], f32, name="g1b", tag="f8")
                for kt in range(KT1):
                    wp_t = wp_pool.tile([P, 4, 512], dt, name="wp_t", tag="wp_t")
                    if mt == 0:
                        if kt == 0:
                            nc.sync.dma_start(wp_t[:, 0:1, :], wp5[0, 0][:, 0:1, :])
                            nc.gpsimd.dma_start(wp_t[:, 1:2, :], wp5[0, 0][:, 1:2, :])
                            nc.sync.dma_start(wp_t[:, 2:3, :], wp5[0, 0][:, 2:3, :])
                            nc.gpsimd.dma_start(wp_t[:, 3:4, :], wp5[0, 0][:, 3:4, :])
                        else:
                            nc.sync.dma_start(wp_t[:, 0:2, :], wp5[mt, kt][:, 0:2, :])
                            nc.sync.dma_start(wp_t[:, 2:4, :], wp5[mt, kt][:, 2:4, :])
                    else:
                        nc.sync.dma_start(wp_t[:], wp5[mt, kt])
                    if mt == 0 and kt + 1 < KT1:
                        j = kt + 1
                        nc.scalar.dma_start(a_res[:, j, 0:2, :], a4[j][:, 0:2, :])
                        nc.scalar.dma_start(a_res[:, j, 2:4, :], a4[j][:, 2:4, :])
                    for ki in range(4):
                        for mi in range(M1_SUB):
                            dst = wv[:, mi, :B_LOCAL] if mi < 3 else g1b[:, :B_LOCAL]
                            nc.tensor.matmul(
                                dst,
                                wp_t[:, ki, mi * P : (mi + 1) * P],
                                a_res[:, kt, ki, :],
                                start=(kt == 0 and ki == 0),
                                stop=(kt == KT1 - 1 and ki == 3),
                            )
                for mi in range(M1_SUB):
                    gs = mt * M1_SUB + mi
                    src = wv[:, mi, :B_LOCAL] if mi < 3 else g1b[:, :B_LOCAL]
                    if gs < DS // P:
                        # act_res carries x1/2048: together with wu16's x2048
                        # the fp16-path PSUM lands in TRUE scale, so the wave
                        # evictions are plain 2x-tier copies (tensor_scalar on
                        # PSUM f32 only gets the 1x tier).
                        nc.vector.tensor_scalar_mul(act_res[:, gs, :], src, COMBINE)
                    else:
                        # fp8 evictions ride the ACT engine so the DVE can
                        # start block 0's W-combos the moment GEMM1 ends.
                        s = gs - DS // P
                        nc.scalar.copy(act8_res[:, s // 2, s % 2, :], src)
                # A-side combos as soon as both D-half chunks exist:
                # chunk kt needs act chunks kt (mt = kt//4) and 12+kt (mt = 3+kt//4)
                if 3 <= mt <= 5:
                    for kt3 in range(4 * (mt - 3), 4 * (mt - 2)):
                        a11 = act_res[:, kt3, 0:P]
                        a12 = act_res[:, KT3 + kt3, 0:P]
                        a21 = act_res[:, kt3, P:B_LOCAL]
                        a22 = act_res[:, KT3 + kt3, P:B_LOCAL]
                        nc.vector.tensor_tensor(sa_res[:, 0, kt3], a21, a22, ADD)
                        nc.vector.tensor_tensor(
                            sa_res[:, 1, kt3], sa_res[:, 0, kt3], a11, SUB
                        )
                        nc.vector.tensor_tensor(sa_res[:, 2, kt3], a11, a21, SUB)
                        nc.vector.tensor_tensor(
                            sa_res[:, 3, kt3], a12, sa_res[:, 1, kt3], SUB
                        )

            # ---------------- GEMM2: Winograd-Strassen + fp8 ----------------
            prev = None  # deferred combine closure state from block j-1

            ustate = {}

            def emit_combines_a(st):
                # First half of j-1's combines: 4 DVE ops. Split in two
                # insertions so the DVE's q-combo runway never dips by more
                # than ~1.3us at once.
                (m_a, m_b, m_c, jj) = st
                P1, P2 = m_a[:, 0], m_a[:, 1]
                P5, P6, P7 = (m_b[:, i] for i in range(3))
                u = temps.tile([P, 7, 512], dt, name="u", tag="u")
                # U1 = P1+P6; U2 = U1+P7; U3 = U1+P5
                nc.vector.tensor_tensor(u[:, 0], P1, P6, ADD)
                nc.vector.tensor_tensor(u[:, 1], u[:, 0], P7, ADD)
                nc.vector.tensor_tensor(u[:, 2], u[:, 0], P5, ADD)
                nc.vector.tensor_tensor(u[:, 3], P1, P2, ADD)       # C11 core
                ustate[jj] = u

            def emit_combines(st, last=False):
                (m_a, m_b, m_c, jj) = st
                P1, P2, P3 = (m_a[:, i] for i in range(3))
                P5, P6, P7 = (m_b[:, i] for i in range(3))
                P4f, F811, F821, F812, F822 = (m_c[:, i] for i in range(5))
                u = ustate.pop(jj, None)
                if u is None:
                    emit_combines_a(st)
                    u = ustate.pop(jj)
                ot = ot_pool.tile([P, 4, 512], dt, name="ot", tag="ot")
                jL = jj * 512
                jR = 8192 + jj * 512
                nc.vector.tensor_tensor(u[:, 4], u[:, 2], P3, ADD)  # C12 core
                nc.vector.tensor_tensor(u[:, 5], u[:, 1], P4f, SUB)  # C21 core
                nc.vector.tensor_tensor(u[:, 6], u[:, 1], P5, ADD)  # C22 core
                # final fp8 adds + out stores ride GpSimd (it is idle and this
                # keeps the DVE under the PE period); the LAST blocks use the
                # DVE instead -- gpsimd's slow adds would serialize the tail --
                # and the very last block fans its stores across four queues
                # so the trigger processing and DMA drain overlap.
                eng = nc.vector if last else nc.gpsimd
                # both B-row tiles of an F-column block store as ONE strided
                # DMA: 32 gpsimd descriptors instead of 64 halves the serial
                # completion-retirement in the final queue drain (~100ns each).
                dqL, dqR = (
                    (nc.gpsimd, nc.sync) if jj == NJ - 1 else (nc.gpsimd, nc.gpsimd)
                )
                eng.tensor_tensor(ot[:, 0], u[:, 3], F811, ADD)
                eng.tensor_tensor(ot[:, 1], u[:, 5], F821, ADD)
                dqL.dma_start(out3[:, :, jL : jL + 512], ot[:, 0:2])
                eng.tensor_tensor(ot[:, 2], u[:, 4], F812, ADD)
                eng.tensor_tensor(ot[:, 3], u[:, 6], F822, ADD)
                dqR.dma_start(out3[:, :, jR : jR + 512], ot[:, 2:4])

            qbuf = {}

            def emit_q(jj, kt):
                # One (j, kt) W tile + its Winograd combo chain on the DVE.
                w_t = w2_pool.tile([P, 4, 512], dt, name="w_t", tag="w_t")
                nc.sync.dma_start(w_t[:], wu16_5[jj, kt])
                q = q_pool.tile([P, 4, 512], dt, name="q", tag="q")
                # q: 0=T1=w12-w11, 1=T2=w22-T1, 2=T3=w22-w12, 3=T4=T2-w21
                nc.vector.tensor_tensor(q[:, 0], w_t[:, 1], w_t[:, 0], SUB)
                nc.vector.tensor_tensor(q[:, 1], w_t[:, 3], q[:, 0], SUB)
                nc.vector.tensor_tensor(q[:, 2], w_t[:, 3], w_t[:, 1], SUB)
                nc.vector.tensor_tensor(q[:, 3], q[:, 1], w_t[:, 2], SUB)
                qbuf[(jj, kt)] = (w_t, q)

            # warm the combo pipeline before block 0 so the GEMM1->GEMM2
            # transition starts with a full q runway
            for kt0 in range(4):
                emit_q(0, kt0)

            for j in range(NJ):
                wvA = psum.tile([P, 3, 512], f32, name="wvA", tag="wv")
                wvB = psum.tile([P, 3, 512], f32, name="wvB", tag="wv")
                p4t = psum.tile([P, 512], f32, name="p4t", tag="f8")
                m_c = m_pool.tile([P, 5, 512], dt, name="m_c", tag="m_c")
                wu8L = wu8_pool.tile([P, PAIRS, 2, 512], f8, name="wu8L", tag="wu8L")
                nc.scalar.dma_start(wu8L[:], wu8_5[j])
                wu8R = wu8_pool.tile([P, PAIRS, 2, 512], f8, name="wu8R", tag="wu8R")
                nc.scalar.dma_start(wu8R[:], wu8_5[NJ + j])

                for kt in range(KT3):
                    if (j, kt) not in qbuf:
                        emit_q(j, kt)
                    w_t, q = qbuf.pop((j, kt))
                    # deferred combines of j-1 go mid-stream on the DVE queue
                    # in two halves: ready to run early in j, done before j's
                    # q's are needed
                    if prev is not None:
                        if kt == 4:
                            emit_combines_a(prev)
                        elif kt == 8:
                            emit_combines(prev, last=(j >= NJ - 1))

                    st = kt == 0
                    sp = kt == KT3 - 1
                    # plain-W products first, then by q chain depth, so the PE
                    # is never waiting on a combo that is still in the DVE pipe
                    nc.tensor.matmul(wvA[:, 0], act_res[:, kt, 0:P], w_t[:, 0],
                                     start=st, stop=sp)      # P1 = A11 B11
                    nc.tensor.matmul(wvA[:, 1], act_res[:, KT3 + kt, 0:P],
                                     w_t[:, 2], start=st, stop=sp)  # P2 = A12 B21
                    nc.tensor.matmul(wvA[:, 2], sa_res[:, 3, kt], w_t[:, 3],
                                     start=st, stop=sp)      # P3 = S4 B22
                    nc.tensor.matmul(wvB[:, 0], sa_res[:, 0, kt], q[:, 0],
                                     start=st, stop=sp)      # P5 = S1 T1
                    nc.tensor.matmul(wvB[:, 1], sa_res[:, 1, kt], q[:, 1],
                                     start=st, stop=sp)      # P6 = S2 T2
                    nc.tensor.matmul(wvB[:, 2], sa_res[:, 2, kt], q[:, 2],
                                     start=st, stop=sp)      # P7 = S3 T3
                    nc.tensor.matmul(p4t[:], act_res[:, KT3 + kt, P:B_LOCAL],
                                     q[:, 3], start=st, stop=sp)  # P4 = A22 T4

                # Prefetch the NEXT block's first two combo tiles ahead of the
                # wave evictions in the DVE stream: the evictions gate on the
                # kt=11 matmuls, and without this the DVE idles ~2us at every
                # block boundary, delivering j+1's combos late.
                if j + 1 < NJ:
                    emit_q(j + 1, 0)
                    emit_q(j + 1, 1)
                # whole-wave single-op evictions on the DVE (true scale ->
                # plain copies at the 2x tier)
                m_a = m_pool.tile([P, 3, 512], dt, name="m_a", tag="m_a")
                nc.vector.tensor_copy(m_a[:], wvA[:])
                m_b = m_pool.tile([P, 3, 512], dt, name="m_b", tag="m_b")
                nc.vector.tensor_copy(m_b[:], wvB[:])
                # P4 + the four fp8 tiles rotate through the two "f8" banks;
                # their scalar-engine evictions stagger so each tile's MMs
                # start exactly as the previous tile's bank frees. P4 is a
                # true-scale fp16-path product (plain copy); the fp8 tiles
                # still carry x2048 and descale here.
                nc.scalar.copy(m_c[:, 0], p4t[:])

                f8specs = [
                    (1, 0, wu8L, 0, P),         # F8_11: out (B1, jL)
                    (2, 1, wu8L, P, B_LOCAL),   # F8_21: out (B2, jL)
                    (3, 0, wu8R, 0, P),         # F8_12: out (B1, jR)
                    (4, 1, wu8R, P, B_LOCAL),   # F8_22: out (B2, jR)
                ]
                for slot, _mi, wu8t, c0, c1 in f8specs:
                    f8t = psum.tile([P, 512], f32, name="f8t", tag="f8")
                    for pr in range(PAIRS):
                        nc.tensor.matmul(
                            f8t[:], act8_res[:, pr, :, c0:c1], wu8t[:, pr],
                            start=(pr == 0), stop=(pr == PAIRS - 1),
                            perf_mode=mybir.MatmulPerfMode.DoubleRow,
                        )
                    nc.scalar.mul(m_c[:, slot], f8t[:], COMBINE)

                prev = (m_a, m_b, m_c, j)

            emit_combines(prev, last=True)
    nc.compile()
    return nc


_NC_CACHE = {}


def _get_nc():
    if "nc" not in _NC_CACHE:
        _NC_CACHE["nc"] = build_nc()
    return _NC_CACHE["nc"]


def _pack_weights(W_prev, W_up):
    KT1 = K_PREV // 512
    M1_TILES = D // 512
    NBLKS = F // 512

    wp = (
        W_prev.reshape(KT1, 4, P, M1_TILES, 512)
        .transpose(3, 0, 2, 1, 4)
        .reshape(M1_TILES * KT1 * P, 4 * 512)
    )
    wp = np.ascontiguousarray(wp)

    # Strassen tiles: wu16[(j*12 + kt)*128 + p, blk*512 + n], blk = dh*2 + fh:
    #   = W2[(dh*12 + kt)*128 + p, fh*8192 + j*512 + n],  W2 = W_up[:DS]*2048
    W2 = W_up[:DS] * np.float16(W_SCALE)
    wu16 = (
        W2.reshape(2, KT3, P, 2, NJ, 512)
        .transpose(4, 1, 2, 0, 3, 5)
        .reshape(NJ * KT3 * P, 4 * 512)
    )
    wu16 = np.ascontiguousarray(wu16)

    q8 = (W_up[DS:] * np.float16(W_SCALE)).astype(F8)
    wu8 = (
        q8.reshape(PAIRS, 2, P, NBLKS, 512)
        .transpose(3, 2, 0, 1, 4)
        .reshape(NBLKS * P, PAIRS * 2 * 512)
    )
    wu8 = np.ascontiguousarray(wu8)
    return wp, wu16, wu8


def run(A_prev, W_prev, W_up, **spmd_kwargs):
    A_prev = np.asarray(A_prev, dtype=np.float16)
    W_prev = np.asarray(W_prev, dtype=np.float16)
    W_up = np.asarray(W_up, dtype=np.float16)
    wp, wu16, wu8 = _pack_weights(W_prev, W_up)
    KT1 = K_PREV // 512
    in_maps = []
    for r in range(N_CORES):
        a_loc = A_prev[r * B_LOCAL : (r + 1) * B_LOCAL, :].T
        a_pk = np.ascontiguousarray(
            a_loc.reshape(KT1, 4, P, B_LOCAL)
            .transpose(0, 2, 1, 3)
            .reshape(KT1 * P, 4 * B_LOCAL)
        )
        in_maps.append({"a_t": a_pk, "w_prev": wp, "wu16": wu16, "wu8": wu8})
    nc = _get_nc()
    res = run_bass_kernel_spmd(
        nc, in_maps, core_ids=list(range(N_CORES)), **spmd_kwargs
    )
    out = np.concatenate([res.results[r]["out"] for r in range(N_CORES)], axis=0)
    return out, res


def kernel(A_prev, W_prev, W_up):
    return run(A_prev, W_prev, W_up)[0]
